# revision 41
# baseline (speedup 1.0000x reference)
"""CRPS loss kernel for Trainium2, 8 NeuronCores.

Math (reference):
  term1 = mean_m |preds - target|                  (B,T,H,W)
  term2 = 0.5 * mean_{i,j} |preds_i - preds_j|     (B,T,H,W)
  crps  = mean_t(term1 - term2)                    (B,H,W)
  pen   = mean_{t<T-1,m} |preds[t+1]-preds[t]|     (B,H,W)
  out   = mean_{b,h,w}(crps + 0.1*pen)             scalar

Everything is a weighted sum of |pairwise differences|, so the final scalar
decomposes into three global sums of absolute differences:
  S1 = sum |p - y|  over (b,t,m,h,w)               weight +1/(B*T*M*H*W)
  S2 = sum_{i<j} |p_i - p_j| over (b,t,h,w)        weight -1/(B*T*M^2*H*W)
  S3 = sum |p[t+1]-p[t]| over (b,t<T-1,m,h,w)      weight +0.1/(B*(T-1)*M*H*W)

Kernel strategy (per core, H sharded 8 ways -> 16 rows each):
  - SBUF layout: partition = (t_local, m) "m-major", free = positions (h,w).
  - GPSIMD casting DMAs load the f32 inputs directly as bf16 (SWDGE cast).
  - TensorE matmuls with constant +-1 bf16 weights generate ALL difference
    streams (pairwise / term1-vs-target / temporal) into PSUM f32 at
    1 column/cycle.
  - ScalarE (activation Abs + accum_out) and VectorE (tensor_reduce with
    apply_absolute_value) alternate strictly over [128, 1024] PSUM tiles
    (4-slot rotation), producing per-partition partial sums into
    accumulator columns; the bulk of each accumulator is DMA'd out
    mid-kernel so only a small remainder DMA trails the last consumer.
  - Accumulators DMA'd out raw; host applies per-(partition, column)
    signed scales in float64 and reduces across cores.
"""

import os
import sys

import numpy as np

try:
    import concourse.bass as bass
except ImportError:  # pragma: no cover - path fallback for fresh environments
    for _p in ("/opt/trn_rl_repo", "/root/.axon_site/_ro/trn_rl_repo"):
        if os.path.isdir(_p):
            sys.path.insert(0, _p)
            break
    import concourse.bass as bass

import ml_dtypes

import concourse.bacc as bacc
from concourse import mybir
from concourse.bass import ts
from concourse.bass_utils import run_bass_kernel_spmd
from concourse.tile import TileContext

F32 = mybir.dt.float32
BF16 = mybir.dt.bfloat16

B, T, M, H, W = 2, 8, 16, 128, 256
NCORES = 8
HC = H // NCORES          # 16 rows of H per core
NPOS = HC * W             # 4096 positions per (b, t) per core
CHUNK = 512               # matmul moving free dim
NCHUNK = NPOS // CHUNK    # 8
TEMPORAL_LAMBDA = 0.1

KG = (68, 84)             # rhs partition rows used by group 0 / group 1
NMAT = 5                  # weight matrices per group (4 pairwise + 1 mixed)
NCOL = 48                 # accumulator columns per engine (80 consumer ops)
SPLIT = 32                # accumulator columns DMA'd out mid-kernel

# consumer cost estimates (ns, from InstructionCostModel) for the greedy
# ACT/DVE assignment of [128, 1024] PSUM consumer ops
COST_ACT = 1108.0
COST_DVE = 1072.0

_CACHE = {}


def _build_weights():
    """Weight matrices Wg0 [68, 5, 128], Wg1 [84, 5, 128], entries in {-1,0,1}.

    Group g covers time slabs t = 4g..4g+3 (local tl = 0..3).
    rhs rows: 16*tl + m for preds, 64 + tl for target[tl],
    g1 only: 68 + m for the slab t=3 copy (for the temporal (3,4) pair).

    mats 0..3: pairwise columns. Linear pairwise index q in [0,480):
      q = 120*tl + pair_index(i<j); mat = q // 120, partition = q % 120.
    mat 4 (mixed): p in [0,64): term1 (p = 16*tl + m)
                   p in [64,112): temporal (p-64 = 16*pl + m, pairs (pl,pl+1))
                   g1 p in [112,128): temporal (3,4) (m = p-112)
    """
    wg = []
    for g in range(2):
        K = KG[g]
        w = np.zeros((K, NMAT, 128), dtype=np.float32)
        q = 0
        for tl in range(4):
            for i in range(M):
                for j in range(i + 1, M):
                    mat, p = divmod(q, 120)
                    w[16 * tl + i, mat, p] += 1.0
                    w[16 * tl + j, mat, p] -= 1.0
                    q += 1
        assert q == 480
        for tl in range(4):
            for m in range(M):
                p = 16 * tl + m
                w[16 * tl + m, 4, p] += 1.0
                w[64 + tl, 4, p] -= 1.0
        for pl in range(3):
            for m in range(M):
                p = 64 + 16 * pl + m
                w[16 * (pl + 1) + m, 4, p] += 1.0
                w[16 * pl + m, 4, p] -= 1.0
        if g == 1:
            for m in range(M):
                p = 112 + m
                w[m, 4, p] += 1.0
                w[68 + m, 4, p] -= 1.0
        wg.append(w.astype(ml_dtypes.bfloat16))
    return wg


def _build_kernel():
    """Returns (nc, col_meta) where col_meta[engine] is a list of (kind, g)."""
    nc = bacc.Bacc("TRN2", target_bir_lowering=False, debug=False)
    preds = nc.declare_dram_parameter("preds", [B, T, M, HC, W], F32, isOutput=False)
    target = nc.declare_dram_parameter("target", [B, T, HC, W], F32, isOutput=False)
    wg0 = nc.declare_dram_parameter("wg0", [KG[0], NMAT, 128], BF16, isOutput=False)
    wg1 = nc.declare_dram_parameter("wg1", [KG[1], NMAT, 128], BF16, isOutput=False)
    acc_out = nc.declare_dram_parameter("acc", [2, 128, NCOL], F32, isOutput=True)

    col_meta = {"act": [], "dve": []}

    with TileContext(nc) as tc:
        with (
            tc.tile_pool(name="data", bufs=1) as data_pool,
            tc.tile_pool(name="scratch", bufs=2) as scratch_pool,
            tc.tile_pool(name="psum", bufs=4, space="PSUM") as psum_pool,
        ):
            wt = [
                data_pool.tile([KG[0], NMAT, 128], BF16, tag="wg0", name="wt0"),
                data_pool.tile([KG[1], NMAT, 128], BF16, tag="wg1", name="wt1"),
            ]
            nc.sync.dma_start(out=wt[0][:], in_=wg0[:])
            nc.sync.dma_start(out=wt[1][:], in_=wg1[:])

            # rhs tiles, loaded bf16 via GPSIMD casting DMAs
            R = [[None, None], [None, None]]
            for b in range(B):
                for g in range(2):
                    r = data_pool.tile(
                        [KG[g], NPOS], BF16, tag=f"r{b}{g}", name=f"r{b}{g}"
                    )
                    src = preds[b, 4 * g : 4 * g + 4].rearrange(
                        "t m h w -> (t m) (h w)"
                    )
                    nc.gpsimd.dma_start(out=r[0:64, :], in_=src)
                    tsrc = target[b, 4 * g : 4 * g + 4].rearrange(
                        "t h w -> t (h w)"
                    )
                    nc.gpsimd.dma_start(out=r[64:68, :], in_=tsrc)
                    R[b][g] = r
                # slab t=3 (bf16) copy for temporal (3,4): SBUF -> SBUF
                nc.sync.dma_start(
                    out=R[b][1][68:84, :], in_=R[b][0][48:64, :]
                )

            acc_act = data_pool.tile([128, NCOL], F32, tag="acc_act", name="acc_act")
            acc_dve = data_pool.tile([128, NCOL], F32, tag="acc_dve", name="acc_dve")
            # both memsets on DVE: Pool's queue is busy with the casting
            # DMAs for ~12us, which would stall DVE's first accumulator write
            nc.vector.memset(acc_act[:], 0.0)
            nc.vector.memset(acc_dve[:], 0.0)

            t_eng = {"n": 0}

            def consume(ptile, kind, g):
                # strict ACT/DVE alternation keeps the 4-slot psum rotation
                # perfectly regular (measured better than cost-greedy)
                e = "act" if t_eng["n"] % 2 == 0 else "dve"
                t_eng["n"] += 1
                if e == "act":
                    j = len(col_meta["act"])
                    if j == SPLIT:
                        # bulk of the accumulator leaves mid-kernel; only a
                        # small remainder DMA sits after the last consumer
                        nc.sync.dma_start(
                            out=acc_out[0, :, 0:SPLIT], in_=acc_act[:, 0:SPLIT]
                        )
                    dummy = scratch_pool.tile(
                        [128, 1024], BF16, tag="dummy", name="dummy"
                    )
                    nc.scalar.activation(
                        out=dummy[:],
                        in_=ptile[:],
                        func=mybir.ActivationFunctionType.Abs,
                        accum_out=acc_act[:, j : j + 1],
                    )
                else:
                    j = len(col_meta["dve"])
                    if j == SPLIT:
                        nc.gpsimd.dma_start(
                            out=acc_out[1, :, 0:SPLIT], in_=acc_dve[:, 0:SPLIT]
                        )
                    nc.vector.tensor_reduce(
                        out=acc_dve[:, j : j + 1],
                        in_=ptile[:],
                        axis=mybir.AxisListType.X,
                        op=mybir.AluOpType.add,
                        apply_absolute_value=True,
                    )
                col_meta[e].append((kind, g))

            # main loop: supergroups of 2 chunks, [128, 1024] psum tiles
            for b in range(B):
                for g in range(2):
                    K = KG[g]
                    for cg in range(NCHUNK // 2):
                        for ci in range(2):
                            c = 2 * cg + ci
                            rhs_pw = R[b][g][0:64, ts(c, CHUNK)]
                            for half in range(2):
                                pw = psum_pool.tile(
                                    [128, 1024], F32, tag="ps", name="pw"
                                )
                                for mi in range(2):
                                    nc.tensor.matmul(
                                        pw[:, ts(mi, CHUNK)],
                                        wt[g][0:64, 2 * half + mi, :],
                                        rhs_pw,
                                        start=True,
                                        stop=True,
                                    )
                                consume(pw, "pw", g)
                        mix = psum_pool.tile([128, 1024], F32, tag="ps", name="mix")
                        for ci in range(2):
                            c = 2 * cg + ci
                            nc.tensor.matmul(
                                mix[:, ts(ci, CHUNK)],
                                wt[g][:, 4, :],
                                R[b][g][0:K, ts(c, CHUNK)],
                                start=True,
                                stop=True,
                            )
                        consume(mix, "mix", g)

            nc.sync.dma_start(
                out=acc_out[0, :, SPLIT:NCOL], in_=acc_act[:, SPLIT:NCOL]
            )
            nc.gpsimd.dma_start(
                out=acc_out[1, :, SPLIT:NCOL], in_=acc_dve[:, SPLIT:NCOL]
            )

    nc.compile()
    return nc, col_meta


def _scale_vectors():
    """Per-partition signed scales for each (kind, g) consumer column."""
    s_pw = 1.0 / (B * T * M * M * H * W)
    s_t1 = 1.0 / (B * T * M * H * W)
    s_tmp = TEMPORAL_LAMBDA / (B * (T - 1) * M * H * W)
    sc = {}
    v = np.zeros(128)
    v[:120] = -s_pw
    sc[("pw", 0)] = sc[("pw", 1)] = v
    v0 = np.zeros(128)
    v0[:64] = s_t1
    v0[64:112] = s_tmp
    sc[("mix", 0)] = v0
    v1 = np.zeros(128)
    v1[:64] = s_t1
    v1[64:128] = s_tmp
    sc[("mix", 1)] = v1
    return sc


def _get_compiled():
    if "nc" not in _CACHE:
        nc, col_meta = _build_kernel()
        _CACHE["nc"] = nc
        _CACHE["col_meta"] = col_meta
        _CACHE["wg"] = _build_weights()
    return _CACHE["nc"], _CACHE["col_meta"], _CACHE["wg"]


TRACE = False
LAST_RESULT = {}


def kernel(preds, target):
    preds = np.ascontiguousarray(np.asarray(preds, dtype=np.float32))
    target = np.ascontiguousarray(np.asarray(target, dtype=np.float32))
    assert preds.shape == (B, T, M, H, W)
    assert target.shape == (B, T, 1, H, W)

    nc, col_meta, wg = _get_compiled()

    in_maps = []
    for c in range(NCORES):
        h0 = c * HC
        in_maps.append(
            {
                "preds": np.ascontiguousarray(preds[:, :, :, h0 : h0 + HC, :]),
                "target": np.ascontiguousarray(
                    target[:, :, 0, h0 : h0 + HC, :]
                ),
                "wg0": wg[0],
                "wg1": wg[1],
            }
        )

    res = run_bass_kernel_spmd(
        nc, in_maps, list(range(NCORES)), trace=TRACE
    )
    LAST_RESULT["exec_time_ns"] = res.exec_time_ns
    LAST_RESULT["profile_json"] = res.profile_json

    sc = _scale_vectors()
    total = 0.0
    for c in range(NCORES):
        acc = np.asarray(res.results[c]["acc"], dtype=np.float64)
        for ei, ename in enumerate(("act", "dve")):
            meta = col_meta[ename]
            if not meta:
                continue
            svec = np.stack([sc[km] for km in meta], axis=1)  # [128, ncols]
            total += float(np.sum(acc[ei, :, : len(meta)] * svec))
    return np.float32(total)
